# revision 23
# baseline (speedup 1.0000x reference)
"""Fast-weight-sum (causal linear attention) transformer layer on 8 TRN2 cores.

Sharding: data-parallel over batch — BSZ=8 batches, one per NeuronCore, no
collectives. Each core runs the full layer for its batch column of h.

Per-core algorithm (L=1024, D=512, H=8 heads, dh=64, chunk C=128):
  qkv = h @ W_qkv (W_qkv columns pre-permuted on host to [Q|K|V] blocks)
  q <- elu(q)+1            (NOT normalized: the 1/sum_d(q) factor cancels
                            between numerator and denominator)
  k <- (elu(k)+1) / sum_d  (normalized on device)
  Chunked causal linear attention with running state S_h = sum k v^T and
  k_state_h = sum k, both accumulated in PSUM. Per chunk c, head h:
    A^T[s,t]  = k_s . q_t                      (PE, 4 heads share a PSUM bank)
    A^T      *= uppertri(s<=t)                 (one DVE op per 4 heads)
    out[t,:]  = A^T.T @ [v | 1 | 0]            (cols 64/65 accumulate the
              + q_t @ [S | k_state | 1]         denominator and sum_d q)
    S_psum   += k_nat^T @ [v | 1]
  denom = out[:,64] + EPS*out[:,65]; attn = 0.125 * out[:,:64] / denom
  layer_out = attn @ W_o; out = layernorm(h + layer_out) * gamma + beta

Matmuls and most elementwise math run in bf16 (f32 PSUM accumulation and
f32 denominators/statistics); the rel-err gate is 2e-2. GpSimd is kept out
of the steady-state pipeline: its elementwise throughput is poor and it
contends with DVE for SBUF ports.
"""

import numpy as np

import concourse.bass as bass
import concourse.mybir as mybir
import concourse.tile as tile
from concourse import bacc
from concourse.bass_utils import run_bass_kernel_spmd
from concourse.masks import make_identity

L, D, F, H, DH = 1024, 512, 1536, 8, 64
P = 128
NLT = L // P          # 8 l-tiles == chunks
KD = D // P           # 4 contraction tiles of d_model
EPS = 1e-5
LN_EPS = 1e-5
SCALE = 1.0 / np.sqrt(DH)
BF = mybir.dt.bfloat16
F32 = mybir.dt.float32
AX = mybir.AluOpType
ACTF = mybir.ActivationFunctionType

LAST_RESULT = None


class _RR:
    """Round-robin copy placement over DVE and ACT (both reach PSUM)."""

    def __init__(self, nc):
        self.nc = nc
        self.i = 0

    def copy(self, out, in_):
        if self.i % 2:
            self.nc.scalar.copy(out, in_)
        else:
            self.nc.vector.tensor_copy(out, in_)
        self.i += 1


def _build_core_kernel(nc, tc):
    h_d = nc.dram_tensor("h", (L, D), F32, kind="ExternalInput")
    wq_d = nc.dram_tensor("W_qkv", (D, F), F32, kind="ExternalInput")
    wo_d = nc.dram_tensor("W_o", (D, D), F32, kind="ExternalInput")
    gamma_d = nc.dram_tensor("gamma", (D,), F32, kind="ExternalInput")
    beta_d = nc.dram_tensor("beta", (D,), F32, kind="ExternalInput")
    out_d = nc.dram_tensor("out", (L, D), F32, kind="ExternalOutput")

    pc = _RR(nc)

    with (
        tc.tile_pool(name="consts", bufs=1) as consts,
        tc.tile_pool(name="wstage", bufs=2) as wstage,
        tc.tile_pool(name="work", bufs=3) as work,
        tc.tile_pool(name="stage", bufs=2) as stage_pool,
        tc.tile_pool(name="pmm", bufs=2, space="PSUM") as pmm,
        tc.tile_pool(name="pab", bufs=1, space="PSUM") as pab,
        tc.tile_pool(name="ppb", bufs=1, space="PSUM") as ppb,
        tc.tile_pool(name="pstate", bufs=1, space="PSUM") as pstate,
    ):
        # ---------- inputs first: weights and h via casting DMAs ----------
        # The first qkv matmul needs wq_b[kt=0] and hT[lt=0]; issue those
        # transfers before anything else. GpSimd DMAs cast f32->bf16 on the
        # fly (no staging tiles, no vector-engine casts). All transposes sit
        # on the sync queue so their waits never block compute engines.
        wq_b = consts.tile([P, KD, F], BF, tag="wq_b")
        wq_view = wq_d.rearrange("(o p) f -> p o f", p=P)
        h_view = h_d.rearrange("(n p) d -> p n d", p=P)
        h_f32 = consts.tile([P, NLT, D], F32, tag="h_f32")
        h_bf = consts.tile([P, NLT, D], BF, tag="h_bf")
        hT = consts.tile([P, NLT, KD, P], BF, tag="hT")
        wo_b = consts.tile([P, KD, D], BF, tag="wo_b")
        wo_view = wo_d.rearrange("(o p) f -> p o f", p=P)

        nc.gpsimd.dma_start(wq_b[:, 0], wq_view[:, 0])
        nc.gpsimd.dma_start(h_bf[:, 0], h_view[:, 0])
        nc.sync.dma_start_transpose(hT[:, 0], h_bf[:, 0])
        for kt in range(1, KD):
            nc.gpsimd.dma_start(wq_b[:, kt], wq_view[:, kt])
        for lt in range(1, NLT):
            nc.gpsimd.dma_start(h_bf[:, lt], h_view[:, lt])
            nc.sync.dma_start_transpose(hT[:, lt], h_bf[:, lt])
        for lt in range(NLT):
            nc.sync.dma_start(h_f32[:, lt], h_view[:, lt])
        for kt in range(KD):
            nc.gpsimd.dma_start(wo_b[:, kt], wo_view[:, kt])

        # ---------- constants ----------
        # Causal mask replicated for 4 heads: utri4[s, j, t] = 1 iff s <= t.
        utri4 = consts.tile([P, 4, P], F32, tag="utri4")
        nc.gpsimd.memset(utri4, 0.0)
        nc.gpsimd.affine_select(
            out=utri4, in_=utri4, compare_op=AX.is_gt, fill=1.0,
            base=0, pattern=[[0, 4], [-1, P]], channel_multiplier=1,
        )

        gamma_ap = gamma_d[:]
        gamma_bc = consts.tile([P, D], BF, tag="gamma_bc")
        nc.gpsimd.dma_start(
            gamma_bc,
            bass.AP(tensor=gamma_ap.tensor, offset=gamma_ap.offset,
                    ap=[[0, P]] + list(gamma_ap.ap)),
        )
        beta_ap = beta_d[:]
        beta_bc = consts.tile([P, D], F32, tag="beta_bc")
        nc.sync.dma_start(
            beta_bc,
            bass.AP(tensor=beta_ap.tensor, offset=beta_ap.offset,
                    ap=[[0, P]] + list(beta_ap.ap)),
        )
        eps_sb = consts.tile([P, 1], F32, tag="eps_sb")
        nc.vector.memset(eps_sb, LN_EPS)


        # ---------- qkv projection + feature map (bf16 pipeline) ----------
        # q and k share one tensor so a single XBAR transpose per l-tile
        # covers both: qk_sb[:, lt, 0:512] = q, [:, lt, 512:1024] = k.
        qk_sb = consts.tile([P, NLT, 2 * D], BF, tag="qk_sb")
        v_ext = consts.tile([P, NLT, H, DH + 2], BF, tag="v_ext")
        nc.gpsimd.memset(v_ext[:, :, :, DH:DH + 2], 0.0)
        nc.gpsimd.memset(v_ext[:, :, :, DH:DH + 1], 1.0)

        for lt in range(NLT):
            for g in range(3):  # 0=q, 1=k, 2=v
                pm = pmm.tile([P, D], F32, tag="mm")
                for kt in range(KD):
                    nc.tensor.matmul(
                        pm,
                        lhsT=hT[:, lt, kt, :],
                        rhs=wq_b[:, kt, g * D:(g + 1) * D],
                        start=(kt == 0),
                        stop=(kt == KD - 1),
                    )
                if g == 2:
                    nc.scalar.copy(
                        v_ext[:, lt, :, 0:DH],
                        pm.rearrange("p (h e) -> p h e", e=DH),
                    )
                else:
                    # elu(x)+1 == relu(x) + min(exp(x), 1)
                    e1 = work.tile([P, D], BF, tag="fmap_e")
                    nc.scalar.activation(e1, pm, ACTF.Exp)
                    nc.vector.tensor_scalar_min(e1, e1, 1.0)
                    r1 = work.tile([P, D], BF, tag="fmap_r")
                    nc.scalar.activation(r1, pm, ACTF.Relu)
                    if g == 0:
                        nc.vector.tensor_add(out=qk_sb[:, lt, 0:D], in0=e1, in1=r1)
                    else:
                        kk = work.tile([P, D], BF, tag="fmap_k")
                        nc.vector.tensor_add(out=kk, in0=e1, in1=r1)
                        ksum = work.tile([P, H], F32, tag="ksum")
                        nc.vector.reduce_sum(
                            out=ksum,
                            in_=kk.rearrange("p (h e) -> p h e", e=DH),
                            axis=mybir.AxisListType.X,
                        )
                        krec = work.tile([P, H], F32, tag="krec")
                        nc.vector.reciprocal(krec, ksum)
                        krec_b = work.tile([P, H], BF, tag="krec_b")
                        nc.vector.tensor_copy(krec_b, krec)
                        nc.vector.tensor_tensor(
                            qk_sb[:, lt, D:2 * D].rearrange("p (h e) -> p h e", e=DH),
                            kk.rearrange("p (h e) -> p h e", e=DH),
                            krec_b[:, :, None].to_broadcast((P, H, DH)),
                            AX.mult,
                        )

        # ---------- q^T, k^T via one XBAR DMA transpose per l-tile ----------
        qkT = consts.tile([P, NLT, 8, P], BF, tag="qkT")
        for lt in range(NLT):
            nc.sync.dma_start_transpose(qkT[:, lt], qk_sb[:, lt])

        # ---------- chunked causal linear attention ----------
        # Per-head attention state in one PSUM bank; head h lives at base
        # partition (h%2)*64 to match its qT/kT slice (PE needs equal
        # lhsT/rhs bases), column slot h//2.
        s_ps = pstate.tile([P, 4 * (DH + 1)], F32, tag="s_ps")
        s_ext = consts.tile([P, 4, DH + 2], BF, tag="s_ext")
        nc.gpsimd.memset(s_ext, 0.0)
        nc.gpsimd.memset(s_ext[:, :, DH + 1:DH + 2], 1.0)
        attn = consts.tile([P, NLT, D], BF, tag="attn")

        for c in range(NLT):
            st = stage_pool.tile([P, H, DH + 2], F32, tag="stage")
            for p in range(4):  # head pair (2p, 2p+1); PSUM regions are
                # bank-aligned: base-64 operands require offset-0 regions.
                ab = pab.tile([P, 2, 512], F32, tag="ab", name="ab")
                for j in range(2):
                    h = 2 * p + j
                    ho = j * DH
                    nc.tensor.matmul(ab[:, j, 0:P],
                                     lhsT=qkT[ho:ho + DH, c, 4 + p, :],
                                     rhs=qkT[ho:ho + DH, c, p, :],
                                     start=True, stop=True)
                am = work.tile([P, 2, P], BF, tag="am2", name="am")
                nc.vector.tensor_tensor(am, ab[:, :, 0:P], utri4[:, 0:2, :],
                                        AX.mult)
                pb = ppb.tile([P, 2, 512], F32, tag="pb", name="pb")
                for j in range(2):
                    h = 2 * p + j
                    ho = j * DH
                    qTh = qkT[ho:ho + DH, c, p, :]
                    nc.tensor.matmul(pb[:, j, 0:DH + 2], lhsT=am[:, j, :],
                                     rhs=v_ext[:, c, h, :],
                                     start=True, stop=False)
                    nc.tensor.matmul(pb[:, j, 0:DH + 2], lhsT=qTh,
                                     rhs=s_ext[ho:ho + DH, p, :],
                                     start=False, stop=True)
                    sps = s_ps[ho:ho + DH, p * (DH + 1):(p + 1) * (DH + 1)]
                    nc.tensor.matmul(sps, lhsT=qk_sb[:, c, D + h * DH:D + (h + 1) * DH],
                                     rhs=v_ext[:, c, h, 0:DH + 1],
                                     start=(c == 0), stop=(c == NLT - 1))
                if c < NLT - 1:
                    pc.copy(s_ext[:, p, 0:DH + 1], s_ps[:, p * (DH + 1):(p + 1) * (DH + 1)])
                pc.copy(st[:, 2 * p:2 * p + 2, :], pb[:, :, 0:DH + 2])
            den = work.tile([P, H], F32, tag="den")
            nc.vector.tensor_scalar_mul(den, st[:, :, DH + 1], EPS)
            nc.vector.tensor_add(out=den, in0=den, in1=st[:, :, DH])
            denr = work.tile([P, H], F32, tag="denr")
            nc.vector.reciprocal(denr, den)
            nc.vector.tensor_scalar_mul(denr, denr, SCALE)
            nc.vector.tensor_tensor(
                attn[:, c].rearrange("p (h e) -> p h e", e=DH),
                st[:, :, 0:DH],
                denr[:, :, None].to_broadcast((P, H, DH)),
                AX.mult,
            )

        # ---------- attn^T via XBAR DMA transpose ----------
        attnT = consts.tile([P, NLT, KD, P], BF, tag="attnT")
        for c in range(NLT):
            nc.sync.dma_start_transpose(attnT[:, c], attn[:, c])

        # ---------- output projection + residual + layernorm ----------
        for lt in range(NLT):
            pm = pmm.tile([P, D], F32, tag="mm")
            for kt in range(KD):
                nc.tensor.matmul(pm, lhsT=attnT[:, lt, kt, :],
                                 rhs=wo_b[:, kt], start=(kt == 0),
                                 stop=(kt == KD - 1))
            x = work.tile([P, D], F32, tag="lnx")
            nc.vector.tensor_add(out=x, in0=pm, in1=h_f32[:, lt])
            stats = work.tile([P, nc.vector.BN_STATS_DIM], F32, tag="stats")
            nc.vector.bn_stats(out=stats, in_=x)
            mv = work.tile([P, nc.vector.BN_AGGR_DIM], F32, tag="mv")
            nc.vector.bn_aggr(out=mv, in_=stats)
            std = work.tile([P, 1], F32, tag="std")
            nc.scalar.activation(std, mv[:, 1:2], ACTF.Sqrt, bias=eps_sb, scale=1.0)
            rstd = work.tile([P, 1], F32, tag="rstd")
            nc.vector.reciprocal(rstd, std)
            # nmr = -mean * rstd, so ACT can apply (x - mean)*rstd as
            # Identity(x*rstd + nmr) with per-partition scale/bias.
            nmr = work.tile([P, 1], F32, tag="nmr")
            nc.vector.tensor_scalar(out=nmr, in0=mv[:, 0:1], scalar1=-1.0,
                                    scalar2=rstd, op0=AX.mult, op1=AX.mult)
            xn = work.tile([P, D], BF, tag="xn")
            nc.scalar.activation(xn, x, ACTF.Identity, bias=nmr, scale=rstd)
            xg = work.tile([P, D], BF, tag="xg")
            nc.vector.tensor_tensor(xg, xn, gamma_bc, AX.mult)
            yo = work.tile([P, D], F32, tag="yo")
            nc.vector.tensor_tensor(yo, xg, beta_bc, AX.add)
            nc.sync.dma_start(out_d[lt * P:(lt + 1) * P, :], yo)


_NC_CACHE = {}


def _get_nc():
    if "nc" not in _NC_CACHE:
        nc = bacc.Bacc("TRN2", target_bir_lowering=False, debug=False)
        with tile.TileContext(nc) as tc:
            _build_core_kernel(nc, tc)
        nc.compile()
        _NC_CACHE["nc"] = nc
    return _NC_CACHE["nc"]


def kernel(h, W_qkv, W_o, gamma, beta, trace=False):
    global LAST_RESULT
    h = np.asarray(h, dtype=np.float32)
    W_qkv = np.asarray(W_qkv, dtype=np.float32)
    W_o = np.asarray(W_o, dtype=np.float32)
    gamma = np.asarray(gamma, dtype=np.float32)
    beta = np.asarray(beta, dtype=np.float32)

    # Permute W_qkv columns from per-head [q|k|v] interleave to [Q|K|V] blocks
    # (heads stay in order inside each block).
    w_perm = np.ascontiguousarray(
        W_qkv.reshape(D, H, 3, DH).transpose(0, 2, 1, 3).reshape(D, F)
    )

    nc = _get_nc()
    in_maps = []
    for b in range(8):
        in_maps.append({
            "h": np.ascontiguousarray(h[:, b, :]),
            "W_qkv": w_perm,
            "W_o": W_o,
            "gamma": gamma,
            "beta": beta,
        })
    res = run_bass_kernel_spmd(nc, in_maps, core_ids=list(range(8)), trace=trace)
    LAST_RESULT = res
    return np.stack([res.results[b]["out"] for b in range(8)], axis=1)


# revision 24
# speedup vs baseline: 1.2202x; 1.2202x over previous
"""Fast-weight-sum (causal linear attention) transformer layer on 8 TRN2 cores.

Sharding: data-parallel over batch — BSZ=8 batches, one per NeuronCore, no
collectives. Each core runs the full layer for its batch column of h.

Per-core algorithm (L=1024, D=512, H=8 heads, dh=64, chunk C=128):
  qkv = h @ W_qkv (W_qkv columns pre-permuted on host to [Q|K|V] blocks)
  q <- elu(q)+1            (NOT normalized: the 1/sum_d(q) factor cancels
                            between numerator and denominator)
  k <- (elu(k)+1) / sum_d  (normalized on device)
  Chunked causal linear attention with running state S_h = sum k v^T and
  k_state_h = sum k, both accumulated in PSUM. Per chunk c, head h:
    A^T[s,t]  = k_s . q_t                      (PE, 4 heads share a PSUM bank)
    A^T      *= uppertri(s<=t)                 (one DVE op per 4 heads)
    out[t,:]  = A^T.T @ [v | 1 | 0]            (cols 64/65 accumulate the
              + q_t @ [S | k_state | 1]         denominator and sum_d q)
    S_psum   += k_nat^T @ [v | 1]
  denom = out[:,64] + EPS*out[:,65]; attn = 0.125 * out[:,:64] / denom
  layer_out = attn @ W_o; out = layernorm(h + layer_out) * gamma + beta

Matmuls and most elementwise math run in bf16 (f32 PSUM accumulation and
f32 denominators/statistics); the rel-err gate is 2e-2. GpSimd is kept out
of the steady-state pipeline: its elementwise throughput is poor and it
contends with DVE for SBUF ports.
"""

import numpy as np

import concourse.bass as bass
import concourse.mybir as mybir
import concourse.tile as tile
from concourse import bacc
from concourse.bass_utils import run_bass_kernel_spmd
from concourse.masks import make_identity

L, D, F, H, DH = 1024, 512, 1536, 8, 64
P = 128
NLT = L // P          # 8 l-tiles == chunks
KD = D // P           # 4 contraction tiles of d_model
EPS = 1e-5
LN_EPS = 1e-5
SCALE = 1.0 / np.sqrt(DH)
BF = mybir.dt.bfloat16
F32 = mybir.dt.float32
AX = mybir.AluOpType
ACTF = mybir.ActivationFunctionType

LAST_RESULT = None


class _RR:
    """Round-robin copy placement over DVE and ACT (both reach PSUM)."""

    def __init__(self, nc):
        self.nc = nc
        self.i = 0

    def copy(self, out, in_):
        if self.i % 2:
            self.nc.scalar.copy(out, in_)
        else:
            self.nc.vector.tensor_copy(out, in_)
        self.i += 1


def _build_core_kernel(nc, tc):
    h_d = nc.dram_tensor("h", (L, D), F32, kind="ExternalInput")
    wq_d = nc.dram_tensor("W_qkv", (D, F), F32, kind="ExternalInput")
    wo_d = nc.dram_tensor("W_o", (D, D), F32, kind="ExternalInput")
    gamma_d = nc.dram_tensor("gamma", (D,), F32, kind="ExternalInput")
    beta_d = nc.dram_tensor("beta", (D,), F32, kind="ExternalInput")
    out_d = nc.dram_tensor("out", (L, D), F32, kind="ExternalOutput")

    pc = _RR(nc)

    with (
        tc.tile_pool(name="consts", bufs=1) as consts,
        tc.tile_pool(name="wstage", bufs=2) as wstage,
        tc.tile_pool(name="work", bufs=3) as work,
        tc.tile_pool(name="stage", bufs=2) as stage_pool,
        tc.tile_pool(name="pmm", bufs=2, space="PSUM") as pmm,
        tc.tile_pool(name="pab", bufs=1, space="PSUM") as pab,
        tc.tile_pool(name="ppb", bufs=1, space="PSUM") as ppb,
        tc.tile_pool(name="pstate", bufs=1, space="PSUM") as pstate,
    ):
        # ---------- inputs first: weights and h via casting DMAs ----------
        # The first qkv matmul needs wq_b[kt=0] and hT[lt=0]; issue those
        # transfers before anything else. GpSimd DMAs cast f32->bf16 on the
        # fly (no staging tiles, no vector-engine casts). All transposes sit
        # on the sync queue so their waits never block compute engines.
        wq_b = consts.tile([P, KD, F], BF, tag="wq_b")
        wq_view = wq_d.rearrange("(o p) f -> p o f", p=P)
        h_view = h_d.rearrange("(n p) d -> p n d", p=P)
        h_f32 = consts.tile([P, NLT, D], F32, tag="h_f32")
        h_bf = consts.tile([P, NLT, D], BF, tag="h_bf")
        hT = consts.tile([P, NLT, KD, P], BF, tag="hT")
        wo_b = consts.tile([P, KD, D], BF, tag="wo_b")
        wo_view = wo_d.rearrange("(o p) f -> p o f", p=P)

        wq_st = [wstage.tile([P, F], F32, tag="wstage", name=f"wq_st{kt}")
                 for kt in range(KD)]
        nc.sync.dma_start(wq_st[0], wq_view[:, 0])
        nc.sync.dma_start(h_f32[:, 0], h_view[:, 0])
        for kt in range(1, KD):
            nc.sync.dma_start(wq_st[kt], wq_view[:, kt])
        for lt in range(1, NLT):
            nc.sync.dma_start(h_f32[:, lt], h_view[:, lt])
        for kt in range(KD):
            for j in range(3):
                pc.copy(wq_b[:, kt, j * D:(j + 1) * D],
                        wq_st[kt][:, j * D:(j + 1) * D])
        for lt in range(NLT):
            pc.copy(h_bf[:, lt], h_f32[:, lt])
            nc.sync.dma_start_transpose(hT[:, lt], h_bf[:, lt])
        for kt in range(KD):
            st = wstage.tile([P, D], F32, tag="wostage")
            nc.sync.dma_start(st, wo_view[:, kt])
            pc.copy(wo_b[:, kt], st)

        # ---------- constants ----------
        # Causal mask replicated for 4 heads: utri4[s, j, t] = 1 iff s <= t.
        utri4 = consts.tile([P, 4, P], F32, tag="utri4")
        nc.gpsimd.memset(utri4, 0.0)
        nc.gpsimd.affine_select(
            out=utri4, in_=utri4, compare_op=AX.is_gt, fill=1.0,
            base=0, pattern=[[0, 4], [-1, P]], channel_multiplier=1,
        )

        gamma_ap = gamma_d[:]
        gamma_bc = consts.tile([P, D], BF, tag="gamma_bc")
        nc.gpsimd.dma_start(
            gamma_bc,
            bass.AP(tensor=gamma_ap.tensor, offset=gamma_ap.offset,
                    ap=[[0, P]] + list(gamma_ap.ap)),
        )
        beta_ap = beta_d[:]
        beta_bc = consts.tile([P, D], F32, tag="beta_bc")
        nc.sync.dma_start(
            beta_bc,
            bass.AP(tensor=beta_ap.tensor, offset=beta_ap.offset,
                    ap=[[0, P]] + list(beta_ap.ap)),
        )
        eps_sb = consts.tile([P, 1], F32, tag="eps_sb")
        nc.vector.memset(eps_sb, LN_EPS)


        # ---------- qkv projection + feature map (bf16 pipeline) ----------
        # q and k share one tensor so a single XBAR transpose per l-tile
        # covers both: qk_sb[:, lt, 0:512] = q, [:, lt, 512:1024] = k.
        qk_sb = consts.tile([P, NLT, 2 * D], BF, tag="qk_sb")
        v_ext = consts.tile([P, NLT, H, DH + 2], BF, tag="v_ext")
        nc.gpsimd.memset(v_ext[:, :, :, DH:DH + 2], 0.0)
        nc.gpsimd.memset(v_ext[:, :, :, DH:DH + 1], 1.0)

        for lt in range(NLT):
            for g in range(3):  # 0=q, 1=k, 2=v
                pm = pmm.tile([P, D], F32, tag="mm")
                for kt in range(KD):
                    nc.tensor.matmul(
                        pm,
                        lhsT=hT[:, lt, kt, :],
                        rhs=wq_b[:, kt, g * D:(g + 1) * D],
                        start=(kt == 0),
                        stop=(kt == KD - 1),
                    )
                if g == 2:
                    nc.scalar.copy(
                        v_ext[:, lt, :, 0:DH],
                        pm.rearrange("p (h e) -> p h e", e=DH),
                    )
                else:
                    # elu(x)+1 == relu(x) + min(exp(x), 1)
                    e1 = work.tile([P, D], BF, tag="fmap_e")
                    nc.scalar.activation(e1, pm, ACTF.Exp)
                    nc.vector.tensor_scalar_min(e1, e1, 1.0)
                    r1 = work.tile([P, D], BF, tag="fmap_r")
                    nc.scalar.activation(r1, pm, ACTF.Relu)
                    if g == 0:
                        nc.vector.tensor_add(out=qk_sb[:, lt, 0:D], in0=e1, in1=r1)
                    else:
                        kk = work.tile([P, D], BF, tag="fmap_k")
                        nc.vector.tensor_add(out=kk, in0=e1, in1=r1)
                        ksum = work.tile([P, H], F32, tag="ksum")
                        nc.vector.reduce_sum(
                            out=ksum,
                            in_=kk.rearrange("p (h e) -> p h e", e=DH),
                            axis=mybir.AxisListType.X,
                        )
                        krec = work.tile([P, H], F32, tag="krec")
                        nc.vector.reciprocal(krec, ksum)
                        krec_b = work.tile([P, H], BF, tag="krec_b")
                        nc.vector.tensor_copy(krec_b, krec)
                        nc.vector.tensor_tensor(
                            qk_sb[:, lt, D:2 * D].rearrange("p (h e) -> p h e", e=DH),
                            kk.rearrange("p (h e) -> p h e", e=DH),
                            krec_b[:, :, None].to_broadcast((P, H, DH)),
                            AX.mult,
                        )

        # ---------- q^T, k^T via one XBAR DMA transpose per l-tile ----------
        qkT = consts.tile([P, NLT, 8, P], BF, tag="qkT")
        for lt in range(NLT):
            nc.sync.dma_start_transpose(qkT[:, lt], qk_sb[:, lt])

        # ---------- chunked causal linear attention ----------
        # Per-head attention state in one PSUM bank; head h lives at base
        # partition (h%2)*64 to match its qT/kT slice (PE needs equal
        # lhsT/rhs bases), column slot h//2.
        s_ps = pstate.tile([P, 4 * (DH + 1)], F32, tag="s_ps")
        s_ext = consts.tile([P, 4, DH + 2], BF, tag="s_ext")
        nc.gpsimd.memset(s_ext, 0.0)
        nc.gpsimd.memset(s_ext[:, :, DH + 1:DH + 2], 1.0)
        attn = consts.tile([P, NLT, D], BF, tag="attn")

        for c in range(NLT):
            st = stage_pool.tile([P, H, DH + 2], F32, tag="stage")
            for p in range(4):  # head pair (2p, 2p+1); PSUM regions are
                # bank-aligned: base-64 operands require offset-0 regions.
                ab = pab.tile([P, 2, 512], F32, tag="ab", name="ab")
                for j in range(2):
                    h = 2 * p + j
                    ho = j * DH
                    nc.tensor.matmul(ab[:, j, 0:P],
                                     lhsT=qkT[ho:ho + DH, c, 4 + p, :],
                                     rhs=qkT[ho:ho + DH, c, p, :],
                                     start=True, stop=True)
                am = work.tile([P, 2, P], BF, tag="am2", name="am")
                nc.vector.tensor_tensor(am, ab[:, :, 0:P], utri4[:, 0:2, :],
                                        AX.mult)
                pb = ppb.tile([P, 2, 512], F32, tag="pb", name="pb")
                for j in range(2):
                    h = 2 * p + j
                    ho = j * DH
                    qTh = qkT[ho:ho + DH, c, p, :]
                    nc.tensor.matmul(pb[:, j, 0:DH + 2], lhsT=am[:, j, :],
                                     rhs=v_ext[:, c, h, :],
                                     start=True, stop=False)
                    nc.tensor.matmul(pb[:, j, 0:DH + 2], lhsT=qTh,
                                     rhs=s_ext[ho:ho + DH, p, :],
                                     start=False, stop=True)
                    sps = s_ps[ho:ho + DH, p * (DH + 1):(p + 1) * (DH + 1)]
                    nc.tensor.matmul(sps, lhsT=qk_sb[:, c, D + h * DH:D + (h + 1) * DH],
                                     rhs=v_ext[:, c, h, 0:DH + 1],
                                     start=(c == 0), stop=(c == NLT - 1))
                if c < NLT - 1:
                    pc.copy(s_ext[:, p, 0:DH + 1], s_ps[:, p * (DH + 1):(p + 1) * (DH + 1)])
                pc.copy(st[:, 2 * p:2 * p + 2, :], pb[:, :, 0:DH + 2])
            den = work.tile([P, H], F32, tag="den")
            nc.vector.tensor_scalar_mul(den, st[:, :, DH + 1], EPS)
            nc.vector.tensor_add(out=den, in0=den, in1=st[:, :, DH])
            denr = work.tile([P, H], F32, tag="denr")
            nc.vector.reciprocal(denr, den)
            nc.vector.tensor_scalar_mul(denr, denr, SCALE)
            nc.vector.tensor_tensor(
                attn[:, c].rearrange("p (h e) -> p h e", e=DH),
                st[:, :, 0:DH],
                denr[:, :, None].to_broadcast((P, H, DH)),
                AX.mult,
            )

        # ---------- attn^T via XBAR DMA transpose ----------
        attnT = consts.tile([P, NLT, KD, P], BF, tag="attnT")
        for c in range(NLT):
            nc.sync.dma_start_transpose(attnT[:, c], attn[:, c])

        # ---------- output projection + residual + layernorm ----------
        for lt in range(NLT):
            pm = pmm.tile([P, D], F32, tag="mm")
            for kt in range(KD):
                nc.tensor.matmul(pm, lhsT=attnT[:, lt, kt, :],
                                 rhs=wo_b[:, kt], start=(kt == 0),
                                 stop=(kt == KD - 1))
            x = work.tile([P, D], F32, tag="lnx")
            nc.vector.tensor_add(out=x, in0=pm, in1=h_f32[:, lt])
            stats = work.tile([P, nc.vector.BN_STATS_DIM], F32, tag="stats")
            nc.vector.bn_stats(out=stats, in_=x)
            mv = work.tile([P, nc.vector.BN_AGGR_DIM], F32, tag="mv")
            nc.vector.bn_aggr(out=mv, in_=stats)
            std = work.tile([P, 1], F32, tag="std")
            nc.scalar.activation(std, mv[:, 1:2], ACTF.Sqrt, bias=eps_sb, scale=1.0)
            rstd = work.tile([P, 1], F32, tag="rstd")
            nc.vector.reciprocal(rstd, std)
            # nmr = -mean * rstd, so ACT can apply (x - mean)*rstd as
            # Identity(x*rstd + nmr) with per-partition scale/bias.
            nmr = work.tile([P, 1], F32, tag="nmr")
            nc.vector.tensor_scalar(out=nmr, in0=mv[:, 0:1], scalar1=-1.0,
                                    scalar2=rstd, op0=AX.mult, op1=AX.mult)
            xn = work.tile([P, D], BF, tag="xn")
            nc.scalar.activation(xn, x, ACTF.Identity, bias=nmr, scale=rstd)
            xg = work.tile([P, D], BF, tag="xg")
            nc.vector.tensor_tensor(xg, xn, gamma_bc, AX.mult)
            yo = work.tile([P, D], F32, tag="yo")
            nc.vector.tensor_tensor(yo, xg, beta_bc, AX.add)
            nc.sync.dma_start(out_d[lt * P:(lt + 1) * P, :], yo)


_NC_CACHE = {}


def _get_nc():
    if "nc" not in _NC_CACHE:
        nc = bacc.Bacc("TRN2", target_bir_lowering=False, debug=False)
        with tile.TileContext(nc) as tc:
            _build_core_kernel(nc, tc)
        nc.compile()
        _NC_CACHE["nc"] = nc
    return _NC_CACHE["nc"]


def kernel(h, W_qkv, W_o, gamma, beta, trace=False):
    global LAST_RESULT
    h = np.asarray(h, dtype=np.float32)
    W_qkv = np.asarray(W_qkv, dtype=np.float32)
    W_o = np.asarray(W_o, dtype=np.float32)
    gamma = np.asarray(gamma, dtype=np.float32)
    beta = np.asarray(beta, dtype=np.float32)

    # Permute W_qkv columns from per-head [q|k|v] interleave to [Q|K|V] blocks
    # (heads stay in order inside each block).
    w_perm = np.ascontiguousarray(
        W_qkv.reshape(D, H, 3, DH).transpose(0, 2, 1, 3).reshape(D, F)
    )

    nc = _get_nc()
    in_maps = []
    for b in range(8):
        in_maps.append({
            "h": np.ascontiguousarray(h[:, b, :]),
            "W_qkv": w_perm,
            "W_o": W_o,
            "gamma": gamma,
            "beta": beta,
        })
    res = run_bass_kernel_spmd(nc, in_maps, core_ids=list(range(8)), trace=trace)
    LAST_RESULT = res
    return np.stack([res.results[b]["out"] for b in range(8)], axis=1)


# revision 25
# speedup vs baseline: 1.2202x; 1.0000x over previous
"""Fast-weight-sum (causal linear attention) transformer layer on 8 TRN2 cores.

Sharding: data-parallel over batch — BSZ=8 batches, one per NeuronCore, no
collectives. Each core runs the full layer for its batch column of h.

Per-core algorithm (L=1024, D=512, H=8 heads, dh=64, chunk C=128):
  qkv = h @ W_qkv (W_qkv columns pre-permuted on host to [Q|K|V] blocks)
  q <- elu(q)+1            (NOT normalized: the 1/sum_d(q) factor cancels
                            between numerator and denominator)
  k <- (elu(k)+1) / sum_d  (normalized on device)
  Chunked causal linear attention with running state S_h = sum k v^T and
  k_state_h = sum k, both accumulated in PSUM. Per chunk c, head h:
    A^T[s,t]  = k_s . q_t                      (PE, 4 heads share a PSUM bank)
    A^T      *= uppertri(s<=t)                 (one DVE op per 4 heads)
    out[t,:]  = A^T.T @ [v | 1 | 0]            (cols 64/65 accumulate the
              + q_t @ [S | k_state | 1]         denominator and sum_d q)
    S_psum   += k_nat^T @ [v | 1]
  denom = out[:,64] + EPS*out[:,65]; attn = 0.125 * out[:,:64] / denom
  layer_out = attn @ W_o; out = layernorm(h + layer_out) * gamma + beta

Matmuls and most elementwise math run in bf16 (f32 PSUM accumulation and
f32 denominators/statistics); the rel-err gate is 2e-2. GpSimd is kept out
of the steady-state pipeline: its elementwise throughput is poor and it
contends with DVE for SBUF ports.
"""

import numpy as np

import concourse.bass as bass
import concourse.mybir as mybir
import concourse.tile as tile
from concourse import bacc
from concourse.bass_utils import run_bass_kernel_spmd
from concourse.masks import make_identity

L, D, F, H, DH = 1024, 512, 1536, 8, 64
P = 128
NLT = L // P          # 8 l-tiles == chunks
KD = D // P           # 4 contraction tiles of d_model
EPS = 1e-5
LN_EPS = 1e-5
SCALE = 1.0 / np.sqrt(DH)
BF = mybir.dt.bfloat16
F32 = mybir.dt.float32
AX = mybir.AluOpType
ACTF = mybir.ActivationFunctionType

LAST_RESULT = None


class _RR:
    """Round-robin copy placement over DVE and ACT (both reach PSUM)."""

    def __init__(self, nc):
        self.nc = nc
        self.i = 0

    def copy(self, out, in_):
        if self.i % 2:
            self.nc.scalar.copy(out, in_)
        else:
            self.nc.vector.tensor_copy(out, in_)
        self.i += 1


def _build_core_kernel(nc, tc):
    h_d = nc.dram_tensor("h", (L, D), F32, kind="ExternalInput")
    wq_d = nc.dram_tensor("W_qkv", (D, F), F32, kind="ExternalInput")
    wo_d = nc.dram_tensor("W_o", (D, D), F32, kind="ExternalInput")
    gamma_d = nc.dram_tensor("gamma", (D,), F32, kind="ExternalInput")
    beta_d = nc.dram_tensor("beta", (D,), F32, kind="ExternalInput")
    out_d = nc.dram_tensor("out", (L, D), F32, kind="ExternalOutput")

    pc = _RR(nc)

    with (
        tc.tile_pool(name="consts", bufs=1) as consts,
        tc.tile_pool(name="wstage", bufs=4) as wstage,
        tc.tile_pool(name="work", bufs=3) as work,
        tc.tile_pool(name="stage", bufs=2) as stage_pool,
        tc.tile_pool(name="pmm", bufs=2, space="PSUM") as pmm,
        tc.tile_pool(name="pab", bufs=1, space="PSUM") as pab,
        tc.tile_pool(name="ppb", bufs=1, space="PSUM") as ppb,
        tc.tile_pool(name="pstate", bufs=1, space="PSUM") as pstate,
    ):
        # ---------- inputs first: weights and h via casting DMAs ----------
        # The first qkv matmul needs wq_b[kt=0] and hT[lt=0]; issue those
        # transfers before anything else. GpSimd DMAs cast f32->bf16 on the
        # fly (no staging tiles, no vector-engine casts). All transposes sit
        # on the sync queue so their waits never block compute engines.
        wq_b = consts.tile([P, KD, F], BF, tag="wq_b")
        wq_view = wq_d.rearrange("(o p) f -> p o f", p=P)
        h_view = h_d.rearrange("(n p) d -> p n d", p=P)
        h_f32 = consts.tile([P, NLT, D], F32, tag="h_f32")
        h_bf = consts.tile([P, NLT, D], BF, tag="h_bf")
        hT = consts.tile([P, NLT, KD, P], BF, tag="hT")
        wo_b = consts.tile([P, KD, D], BF, tag="wo_b")
        wo_view = wo_d.rearrange("(o p) f -> p o f", p=P)

        wq_st = [wstage.tile([P, F], F32, tag="wstage", name=f"wq_st{kt}")
                 for kt in range(KD)]
        nc.sync.dma_start(wq_st[0], wq_view[:, 0])
        nc.sync.dma_start(h_f32[:, 0], h_view[:, 0])
        for kt in range(1, KD):
            nc.sync.dma_start(wq_st[kt], wq_view[:, kt])
        for lt in range(1, NLT):
            nc.sync.dma_start(h_f32[:, lt], h_view[:, lt])
        for kt in range(KD):
            for j in range(3):
                pc.copy(wq_b[:, kt, j * D:(j + 1) * D],
                        wq_st[kt][:, j * D:(j + 1) * D])
        for lt in range(NLT):
            pc.copy(h_bf[:, lt], h_f32[:, lt])
            nc.sync.dma_start_transpose(hT[:, lt], h_bf[:, lt])
        for kt in range(KD):
            st = wstage.tile([P, D], F32, tag="wostage")
            nc.sync.dma_start(st, wo_view[:, kt])
            pc.copy(wo_b[:, kt], st)

        # ---------- constants ----------
        # Causal mask replicated for 4 heads: utri4[s, j, t] = 1 iff s <= t.
        utri4 = consts.tile([P, 4, P], F32, tag="utri4")
        nc.gpsimd.memset(utri4, 0.0)
        nc.gpsimd.affine_select(
            out=utri4, in_=utri4, compare_op=AX.is_gt, fill=1.0,
            base=0, pattern=[[0, 4], [-1, P]], channel_multiplier=1,
        )

        gamma_ap = gamma_d[:]
        gamma_bc = consts.tile([P, D], BF, tag="gamma_bc")
        nc.gpsimd.dma_start(
            gamma_bc,
            bass.AP(tensor=gamma_ap.tensor, offset=gamma_ap.offset,
                    ap=[[0, P]] + list(gamma_ap.ap)),
        )
        beta_ap = beta_d[:]
        beta_bc = consts.tile([P, D], F32, tag="beta_bc")
        nc.sync.dma_start(
            beta_bc,
            bass.AP(tensor=beta_ap.tensor, offset=beta_ap.offset,
                    ap=[[0, P]] + list(beta_ap.ap)),
        )
        eps_sb = consts.tile([P, 1], F32, tag="eps_sb")
        nc.vector.memset(eps_sb, LN_EPS)


        # ---------- qkv projection + feature map (bf16 pipeline) ----------
        # q and k share one tensor so a single XBAR transpose per l-tile
        # covers both: qk_sb[:, lt, 0:512] = q, [:, lt, 512:1024] = k.
        qk_sb = consts.tile([P, NLT, 2 * D], BF, tag="qk_sb")
        v_ext = consts.tile([P, NLT, H, DH + 2], BF, tag="v_ext")
        nc.gpsimd.memset(v_ext[:, :, :, DH:DH + 2], 0.0)
        nc.gpsimd.memset(v_ext[:, :, :, DH:DH + 1], 1.0)

        for lt in range(NLT):
            for g in range(3):  # 0=q, 1=k, 2=v
                pm = pmm.tile([P, D], F32, tag="mm")
                for kt in range(KD):
                    nc.tensor.matmul(
                        pm,
                        lhsT=hT[:, lt, kt, :],
                        rhs=wq_b[:, kt, g * D:(g + 1) * D],
                        start=(kt == 0),
                        stop=(kt == KD - 1),
                    )
                if g == 2:
                    nc.scalar.copy(
                        v_ext[:, lt, :, 0:DH],
                        pm.rearrange("p (h e) -> p h e", e=DH),
                    )
                else:
                    # elu(x)+1 == relu(x) + min(exp(x), 1)
                    e1 = work.tile([P, D], BF, tag="fmap_e")
                    nc.scalar.activation(e1, pm, ACTF.Exp)
                    nc.vector.tensor_scalar_min(e1, e1, 1.0)
                    r1 = work.tile([P, D], BF, tag="fmap_r")
                    nc.scalar.activation(r1, pm, ACTF.Relu)
                    if g == 0:
                        nc.vector.tensor_add(out=qk_sb[:, lt, 0:D], in0=e1, in1=r1)
                    else:
                        kk = work.tile([P, D], BF, tag="fmap_k")
                        nc.vector.tensor_add(out=kk, in0=e1, in1=r1)
                        ksum = work.tile([P, H], F32, tag="ksum")
                        nc.vector.reduce_sum(
                            out=ksum,
                            in_=kk.rearrange("p (h e) -> p h e", e=DH),
                            axis=mybir.AxisListType.X,
                        )
                        krec = work.tile([P, H], F32, tag="krec")
                        nc.vector.reciprocal(krec, ksum)
                        krec_b = work.tile([P, H], BF, tag="krec_b")
                        nc.vector.tensor_copy(krec_b, krec)
                        nc.vector.tensor_tensor(
                            qk_sb[:, lt, D:2 * D].rearrange("p (h e) -> p h e", e=DH),
                            kk.rearrange("p (h e) -> p h e", e=DH),
                            krec_b[:, :, None].to_broadcast((P, H, DH)),
                            AX.mult,
                        )

        # ---------- q^T, k^T via one XBAR DMA transpose per l-tile ----------
        qkT = consts.tile([P, NLT, 8, P], BF, tag="qkT")
        for lt in range(NLT):
            nc.sync.dma_start_transpose(qkT[:, lt], qk_sb[:, lt])

        # ---------- chunked causal linear attention ----------
        # Per-head attention state in one PSUM bank; head h lives at base
        # partition (h%2)*64 to match its qT/kT slice (PE needs equal
        # lhsT/rhs bases), column slot h//2.
        s_ps = pstate.tile([P, 4 * (DH + 1)], F32, tag="s_ps")
        s_ext = consts.tile([P, 4, DH + 2], BF, tag="s_ext")
        nc.gpsimd.memset(s_ext, 0.0)
        nc.gpsimd.memset(s_ext[:, :, DH + 1:DH + 2], 1.0)
        attn = consts.tile([P, NLT, D], BF, tag="attn")

        for c in range(NLT):
            st = stage_pool.tile([P, H, DH + 2], F32, tag="stage")
            for p in range(4):  # head pair (2p, 2p+1); PSUM regions are
                # bank-aligned: base-64 operands require offset-0 regions.
                ab = pab.tile([P, 2, 512], F32, tag="ab", name="ab")
                for j in range(2):
                    h = 2 * p + j
                    ho = j * DH
                    nc.tensor.matmul(ab[:, j, 0:P],
                                     lhsT=qkT[ho:ho + DH, c, 4 + p, :],
                                     rhs=qkT[ho:ho + DH, c, p, :],
                                     start=True, stop=True)
                am = work.tile([P, 2, P], BF, tag="am2", name="am")
                nc.vector.tensor_tensor(am, ab[:, :, 0:P], utri4[:, 0:2, :],
                                        AX.mult)
                pb = ppb.tile([P, 2, 512], F32, tag="pb", name="pb")
                for j in range(2):
                    h = 2 * p + j
                    ho = j * DH
                    qTh = qkT[ho:ho + DH, c, p, :]
                    nc.tensor.matmul(pb[:, j, 0:DH + 2], lhsT=am[:, j, :],
                                     rhs=v_ext[:, c, h, :],
                                     start=True, stop=False)
                    nc.tensor.matmul(pb[:, j, 0:DH + 2], lhsT=qTh,
                                     rhs=s_ext[ho:ho + DH, p, :],
                                     start=False, stop=True)
                    sps = s_ps[ho:ho + DH, p * (DH + 1):(p + 1) * (DH + 1)]
                    nc.tensor.matmul(sps, lhsT=qk_sb[:, c, D + h * DH:D + (h + 1) * DH],
                                     rhs=v_ext[:, c, h, 0:DH + 1],
                                     start=(c == 0), stop=(c == NLT - 1))
                if c < NLT - 1:
                    pc.copy(s_ext[:, p, 0:DH + 1], s_ps[:, p * (DH + 1):(p + 1) * (DH + 1)])
                pc.copy(st[:, 2 * p:2 * p + 2, :], pb[:, :, 0:DH + 2])
            den = work.tile([P, H], F32, tag="den")
            nc.vector.tensor_scalar_mul(den, st[:, :, DH + 1], EPS)
            nc.vector.tensor_add(out=den, in0=den, in1=st[:, :, DH])
            denr = work.tile([P, H], F32, tag="denr")
            nc.vector.reciprocal(denr, den)
            nc.vector.tensor_scalar_mul(denr, denr, SCALE)
            nc.vector.tensor_tensor(
                attn[:, c].rearrange("p (h e) -> p h e", e=DH),
                st[:, :, 0:DH],
                denr[:, :, None].to_broadcast((P, H, DH)),
                AX.mult,
            )

        # ---------- attn^T via XBAR DMA transpose ----------
        attnT = consts.tile([P, NLT, KD, P], BF, tag="attnT")
        for c in range(NLT):
            nc.sync.dma_start_transpose(attnT[:, c], attn[:, c])

        # ---------- output projection + residual + layernorm ----------
        for lt in range(NLT):
            pm = pmm.tile([P, D], F32, tag="mm")
            for kt in range(KD):
                nc.tensor.matmul(pm, lhsT=attnT[:, lt, kt, :],
                                 rhs=wo_b[:, kt], start=(kt == 0),
                                 stop=(kt == KD - 1))
            x = work.tile([P, D], F32, tag="lnx")
            nc.vector.tensor_add(out=x, in0=pm, in1=h_f32[:, lt])
            stats = work.tile([P, nc.vector.BN_STATS_DIM], F32, tag="stats")
            nc.vector.bn_stats(out=stats, in_=x)
            mv = work.tile([P, nc.vector.BN_AGGR_DIM], F32, tag="mv")
            nc.vector.bn_aggr(out=mv, in_=stats)
            std = work.tile([P, 1], F32, tag="std")
            nc.scalar.activation(std, mv[:, 1:2], ACTF.Sqrt, bias=eps_sb, scale=1.0)
            rstd = work.tile([P, 1], F32, tag="rstd")
            nc.vector.reciprocal(rstd, std)
            # nmr = -mean * rstd, so ACT can apply (x - mean)*rstd as
            # Identity(x*rstd + nmr) with per-partition scale/bias.
            nmr = work.tile([P, 1], F32, tag="nmr")
            nc.vector.tensor_scalar(out=nmr, in0=mv[:, 0:1], scalar1=-1.0,
                                    scalar2=rstd, op0=AX.mult, op1=AX.mult)
            xn = work.tile([P, D], BF, tag="xn")
            nc.scalar.activation(xn, x, ACTF.Identity, bias=nmr, scale=rstd)
            xg = work.tile([P, D], BF, tag="xg")
            nc.vector.tensor_tensor(xg, xn, gamma_bc, AX.mult)
            yo = work.tile([P, D], F32, tag="yo")
            nc.vector.tensor_tensor(yo, xg, beta_bc, AX.add)
            nc.sync.dma_start(out_d[lt * P:(lt + 1) * P, :], yo)


_NC_CACHE = {}


def _get_nc():
    if "nc" not in _NC_CACHE:
        nc = bacc.Bacc("TRN2", target_bir_lowering=False, debug=False)
        with tile.TileContext(nc) as tc:
            _build_core_kernel(nc, tc)
        nc.compile()
        _NC_CACHE["nc"] = nc
    return _NC_CACHE["nc"]


def kernel(h, W_qkv, W_o, gamma, beta, trace=False):
    global LAST_RESULT
    h = np.asarray(h, dtype=np.float32)
    W_qkv = np.asarray(W_qkv, dtype=np.float32)
    W_o = np.asarray(W_o, dtype=np.float32)
    gamma = np.asarray(gamma, dtype=np.float32)
    beta = np.asarray(beta, dtype=np.float32)

    # Permute W_qkv columns from per-head [q|k|v] interleave to [Q|K|V] blocks
    # (heads stay in order inside each block).
    w_perm = np.ascontiguousarray(
        W_qkv.reshape(D, H, 3, DH).transpose(0, 2, 1, 3).reshape(D, F)
    )

    nc = _get_nc()
    in_maps = []
    for b in range(8):
        in_maps.append({
            "h": np.ascontiguousarray(h[:, b, :]),
            "W_qkv": w_perm,
            "W_o": W_o,
            "gamma": gamma,
            "beta": beta,
        })
    res = run_bass_kernel_spmd(nc, in_maps, core_ids=list(range(8)), trace=trace)
    LAST_RESULT = res
    return np.stack([res.results[b]["out"] for b in range(8)], axis=1)
